# revision 65
# baseline (speedup 1.0000x reference)
"""Grouped-decoder MLP (P=8 experts) on 8 Trainium2 NeuronCores.

Expert-parallel: core p owns decoder p (z replicated). All matmuls fp16
(1 cyc/row on PE, ~2^-11 relative precision), fp32 PSUM accumulation,
fp32 stats math.

Per core:
  phase A: h1_pre = W6 @ zT -> PSUM; DVE bn_stats on PSUM; ACT drains
           PSUM -> h1 (fp16 SBUF, pre-relu).
  stats:   BN1 aggregate -> a1 = g6*rstd1 (folded into W7 -> w7s fp16),
           c1a = be6/a1 - mu1. c1a also written as row 16 of w6aug so
           the transposed matmul adds it in-matmul.
  phase B/C: h1 relu'd in place (DVE, +c1a bias); row sums -> sums1
           (DVE); transposed h1T' computed directly from z via
           mm1T = ztaug^T @ w6aug (bias via augmented ones row), relu
           drained by ACT; C += h1T'^T h1T' accumulated on PE.
  BN2 stats analytically from C (b7 cancels): q2 = rowdot(w7s^T C, w7s),
           m2 = w7s^T mu1'; var2 = q2/N - m2^2. Uses the SAME fp16 w7s
           as mm2 so the analytic stats match the actual h2 exactly.
  phase D (groups of G chunks, stationary-reuse ordering):
           h2 = w7s^T @ h1' -> relu(x + c2/a2) (ACT/DVE split, a2
           folded into W8 -> w8s fp16) -> emT = sigmoid(w8s^T @ h2 +
           b8) -> fp16 DRAM.
Output emT [224, 32768] fp16 per core; host upcasts/transposes/stacks.
"""

import os
import sys

import numpy as np

for _p in ("/opt/trn_rl_repo",):
    if _p not in sys.path and os.path.isdir(_p):
        sys.path.insert(0, _p)

import concourse.bass as bass  # noqa: E402
import concourse.bass_utils as _bu  # noqa: E402
import concourse.tile as tile  # noqa: E402
from concourse import bacc, mybir  # noqa: E402
from concourse.bass import ds, ts  # noqa: E402
from concourse.masks import make_identity  # noqa: E402

def _dedup_ldweights(nc):
    """Drop PE weight reloads whose stationary is already in the array.

    Tile legalization emits one InstLdweights per matmul. When
    consecutive PE matmuls share the same stationary AP (phase-D
    grouped loops, the C/sums matmul pairs, phase-A's constant w6),
    the repeat loads cost ~contraction-rows cycles each for nothing.
    Only waitless, updateless loads are dropped: a load whose
    stationary may have been rewritten carries the RAW-dep wait, so it
    is never removed.
    """
    removed = 0
    for f in nc.m.functions:
        for blk in f.blocks:
            last_sig = None
            keep = []
            for inst in blk.instructions:
                if isinstance(inst, mybir.InstLdweights):
                    si = inst.sync_info
                    clean = si is None or (not si.on_wait and not si.on_update)
                    sig = (
                        str(inst.ins[0]),
                        getattr(inst, "is_transpose", None),
                        getattr(inst, "perf_mode", None),
                    )
                    if clean and sig == last_sig:
                        removed += 1
                        continue
                    last_sig = sig
                elif isinstance(inst, (mybir.InstMatmult, mybir.InstMatmultMx)):
                    pass  # matmuls don't disturb the loaded stationary
                keep.append(inst)
            if len(keep) != len(blk.instructions):
                blk.instructions[:] = keep
    return removed

FP32 = mybir.dt.float32
FP16 = mybir.dt.float16
AF = mybir.ActivationFunctionType
ALU = mybir.AluOpType
AX = mybir.AxisListType

N = 32768
ZD = 16
F1 = 128
F2 = 512
CH = 224
P = 8
EPS = 1e-5
NW = 512          # n-chunk width
NCH = N // NW     # 64 chunks
KC = F2 // 128    # 4 f2/K blocks
CSZ = (128, CH - 128)  # output-channel blocks: 128 + 96
G = 4             # phase-D group size (stationary-weight reuse)

# engine-split knob: fraction of phase-D BN2+ReLU drains on ACT
# (the rest go to DVE); ACT also does all sigmoids.
D_ACT_NUM, D_ACT_DEN = 6, 16


def _pick_act(i, num=D_ACT_NUM, den=D_ACT_DEN):
    return (i * num) // den != ((i + 1) * num) // den


def build_program(n_chunks=NCH):
    n = n_chunks * NW
    nblk = n // 128
    nc = bacc.Bacc("TRN2", target_bir_lowering=False, debug=False)

    ztaug_d = nc.dram_tensor("ztaug", [ZD, n], FP16, kind="ExternalInput").ap()
    z4_d = nc.dram_tensor("z4", [64, n // 4], FP16, kind="ExternalInput").ap()
    w6b_d = nc.dram_tensor("w6b", [97, NW], FP16, kind="ExternalInput").ap()
    w6t_d = nc.dram_tensor("w6t", [ZD, F1], FP16, kind="ExternalInput").ap()
    w7t_d = nc.dram_tensor("w7t", [F1, F2], FP32, kind="ExternalInput").ap()
    w8t_d = nc.dram_tensor("w8t", [KC, 128, CH], FP32, kind="ExternalInput").ap()
    g6_d = nc.dram_tensor("g6", [F1, 1], FP32, kind="ExternalInput").ap()
    be6_d = nc.dram_tensor("be6", [F1, 1], FP32, kind="ExternalInput").ap()
    g7_d = nc.dram_tensor("g7", [KC, 128, 1], FP32, kind="ExternalInput").ap()
    be7_d = nc.dram_tensor("be7", [KC, 128, 1], FP32, kind="ExternalInput").ap()
    b8_d = nc.dram_tensor("b8", [CH, 1], FP32, kind="ExternalInput").ap()
    emt_d = nc.dram_tensor("emt", [CH, n], FP16, kind="ExternalOutput").ap()

    with tile.TileContext(nc) as tc:
        with (
            tc.tile_pool(name="consts", bufs=1) as consts,
            tc.tile_pool(name="h1p", bufs=1) as h1p,
            tc.tile_pool(name="natp", bufs=4) as natp,
            tc.tile_pool(name="h2p", bufs=32) as h2p,
            tc.tile_pool(name="emp", bufs=4) as emp,
            tc.tile_pool(name="smalls", bufs=1) as smalls,
            tc.tile_pool(name="pmm", bufs=3, space="PSUM") as pmm,
            tc.tile_pool(name="ppse", bufs=4, space="PSUM") as ppse,
            tc.tile_pool(name="pc", bufs=1, space="PSUM") as pc,
        ):
            # ---- constants / weights in SBUF ----
            # ztaug has only 17 partitions, so its DMA runs at ~17/128 of
            # peak; chunk it so phase A can start on early chunks while
            # later ones stream in.
            NZD = 16
            zw = n // NZD
            ztaug = consts.tile([ZD, n], FP16)
            w6aug = consts.tile([ZD, F1], FP16)
            nc.sync.dma_start(out=w6aug, in_=w6t_d)
            w6b = consts.tile([97, NW], FP16)
            # sync queue carries only the z stream phase A consumes; the
            # first chunks are small so phase A starts early. Everything
            # else rides the Pool queue.
            zoff = 0
            for zwj in [256, 256, 512, 512, 512] + [2048] * 15:
                nc.sync.dma_start(
                    out=ztaug[:, ds(zoff, zwj)], in_=ztaug_d[:, ds(zoff, zwj)]
                )
                zoff += zwj
            assert zoff == n
            nc.gpsimd.dma_start(out=w6b[0:96, :], in_=w6b_d[0:96])
            # 4-block-packed z (16 rows per block, partitions 0-63) +
            # block-diagonal w6 for the single-matmul transposed mm1T.
            # z4 row 96 is ones and w6b row 96 gets c1a^T tiled 4x after
            # stats (96 is an engine-writable base partition; rows 64-95
            # are zero so they contribute nothing).
            z4 = consts.tile([97, n // 4], FP16)
            nc.gpsimd.memset(z4[64:96, :], 0.0)
            nc.gpsimd.memset(z4[96:97, :], 1.0)
            w7t32 = consts.tile([F1, F2], FP32)
            nc.gpsimd.dma_start(out=w7t32, in_=w7t_d)
            for j in range(8):
                zw4 = n // 4 // 8
                nc.gpsimd.dma_start(out=z4[0:64, ts(j, zw4)], in_=z4_d[:, ts(j, zw4)])
            w8t32 = consts.tile([128, KC, CH], FP32)
            g7 = consts.tile([128, KC], FP32)
            be7 = consts.tile([128, KC], FP32)
            for kc in range(KC):
                nc.gpsimd.dma_start(out=w8t32[:, kc, :], in_=w8t_d[kc])
                nc.gpsimd.dma_start(out=g7[:, kc : kc + 1], in_=g7_d[kc])
                nc.gpsimd.dma_start(out=be7[:, kc : kc + 1], in_=be7_d[kc])
            g6 = consts.tile([F1, 1], FP32)
            nc.gpsimd.dma_start(out=g6, in_=g6_d)
            be6 = consts.tile([F1, 1], FP32)
            nc.gpsimd.dma_start(out=be6, in_=be6_d)
            b8 = consts.tile([128, 2], FP32)
            nc.gpsimd.dma_start(out=b8[:, 0:1], in_=b8_d[0:128])
            nc.gpsimd.dma_start(out=b8[: CSZ[1], 1:2], in_=b8_d[128:CH])
            ident16 = consts.tile([128, 128], FP16)
            make_identity(nc, ident16)
            eps_t = consts.tile([128, 1], FP32)
            nc.vector.memset(eps_t, EPS)
            ones16 = consts.tile([128, 1], FP16)
            nc.vector.memset(ones16, 1.0)

            w7s = consts.tile([F1, F2], FP16)       # a1-folded W7^T, fp16
            w7snT = consts.tile([128, F2], FP16)    # w7s natural form
            w8s = consts.tile([128, KC, CH], FP16)  # a2-folded W8^T, fp16

            h1 = h1p.tile([F1, n], FP16)
            stats6 = smalls.tile([F1, n_chunks, 6], FP32)

            # ---- phase A: h1_pre = W6 @ zT, stats ----
            for k in range(n_chunks):
                ps = pmm.tile([128, NW], FP32, tag="mm")
                nc.tensor.matmul(
                    ps, w6aug, ztaug[:, ts(k, NW)],
                    start=True, stop=True,
                )
                nc.scalar.copy(h1[:, ts(k, NW)], ps)
                if k >= n_chunks - 4:
                    # skip the copy->stats serialization at the phase tail
                    nc.vector.bn_stats(out=stats6[:, k, :], in_=ps)
                else:
                    nc.vector.bn_stats(out=stats6[:, k, :], in_=h1[:, ts(k, NW)])

            # ---- BN1 stats -> a1, c1a; fold a1 into w7s ----
            mv1 = smalls.tile([F1, 2], FP32)
            nc.vector.bn_aggr(out=mv1, in_=stats6)
            a1 = smalls.tile([F1, 1], FP32)
            c1a = smalls.tile([F1, 1], FP32)
            tmp1 = smalls.tile([F1, 1], FP32)
            nc.scalar.activation(tmp1, mv1[:, 1:2], AF.Sqrt, bias=eps_t, scale=1.0)
            nc.vector.reciprocal(tmp1, tmp1)
            nc.vector.tensor_mul(a1, g6, tmp1)
            ra1 = smalls.tile([F1, 1], FP32)
            nc.vector.reciprocal(ra1, a1)
            nc.vector.tensor_mul(ra1, be6, ra1)
            nc.vector.tensor_sub(c1a, ra1, mv1[:, 0:1])
            # c1a^T tiled 4x -> row 64 of w6b (in-matmul bias for mm1T)
            c1a16 = smalls.tile([F1, 1], FP16)
            nc.vector.tensor_copy(c1a16, c1a)
            rowps = ppse.tile([128, NW], FP32, tag="pt")
            nc.tensor.matmul(rowps[0:1, 0:128], c1a16, ident16, start=True, stop=True)
            for q in range(4):
                nc.vector.tensor_copy(
                    w6b[96:97, ts(q, 128)], rowps[0:1, 0:128]
                )
            nc.vector.tensor_scalar_mul(w7s, w7t32, a1)
            # w7s natural form via PE transposes (same fp16 values as mm2)
            for kc in range(KC):
                tp = ppse.tile([128, NW], FP16, tag="pt")
                nc.tensor.transpose(tp[:, 0:128], w7s[:, ts(kc, 128)], ident16)
                nc.vector.tensor_copy(w7snT[:, ts(kc, 128)], tp[:, 0:128])

            # ---- phase B/C: relu h1 in place; C = sum h1T'^T h1T' ----
            # cps cols 0:128 accumulate C; col 128 accumulates sum(h1')
            # via a ones-column matmul sharing the C matmul's stationary.
            # C matmuls for chunk k-2 are emitted after mm1T of chunk k
            # so the PE never waits on the ACT relu-drain in between.
            SKEW = 2
            cps = pc.tile([128, 136], FP32)
            hts = {}
            for k in range(n_chunks + SKEW):
                if k < n_chunks:
                    hc = h1[:, ts(k, NW)]
                    nc.vector.tensor_scalar(hc, hc, c1a, 0.0, ALU.add, ALU.max)
                    pst = ppse.tile([128, NW], FP32, tag="pt", name=f"pst_{k}")
                    nc.tensor.matmul(
                        pst, z4[:, ts(k, 128)], w6b, start=True, stop=True
                    )
                    ht = natp.tile([128, NW], FP16, tag="nat", name=f"ht_{k}")
                    if k % 4 == 3:
                        nc.vector.tensor_scalar(ht, pst, 0.0, 0.0, ALU.add, ALU.max)
                    else:
                        nc.scalar.activation(ht, pst, AF.Relu, scale=1.0)
                    hts[k] = ht
                kc_ = k - SKEW
                if kc_ >= 0:
                    ht = hts.pop(kc_)
                    for j in range(NW // 128):
                        b = kc_ * (NW // 128) + j
                        nc.tensor.matmul(
                            cps[:, 0:128], ht[:, ts(j, 128)], ht[:, ts(j, 128)],
                            start=(b == 0), stop=(b == nblk - 1),
                        )
                        nc.tensor.matmul(
                            cps[:, 128:129], ht[:, ts(j, 128)], ones16,
                            start=(b == 0), stop=(b == nblk - 1),
                        )

            # ---- BN2 statistics from C ----
            # c16 pre-scaled by 1/n so qs comes out as q2/n directly
            mu16 = smalls.tile([F1, 1], FP16)
            nc.scalar.mul(mu16, cps[:, 128:129], 1.0 / n)
            c16 = smalls.tile([128, 128], FP16)
            nc.scalar.mul(c16, cps[:, 0:128], 1.0 / n)

            qs = smalls.tile([128, KC], FP32)
            m2a = smalls.tile([128, KC], FP32)
            c2a = smalls.tile([128, KC], FP32)
            scratch = smalls.tile([128, KC, 128], FP32)
            # all four kc blocks land in one psum tile (m2 columns go to
            # spare cps columns) so the mul/reduce/copy run once, wide
            e2all = ppse.tile([128, NW], FP32, tag="pt")
            for kc in range(KC):
                nc.tensor.matmul(
                    e2all[:, ts(kc, 128)], w7s[:, ts(kc, 128)], c16,
                    start=True, stop=True,
                )
                nc.tensor.matmul(
                    cps[:, 130 + kc : 131 + kc], w7s[:, ts(kc, 128)], mu16,
                    start=True, stop=True,
                )
            nc.vector.tensor_mul(scratch, e2all, w7snT)
            nc.vector.tensor_reduce(out=qs, in_=scratch, axis=AX.X, op=ALU.add)
            nc.vector.tensor_copy(m2a, cps[:, 130:134])
            # batched [128, KC] scalar chain
            m2sq = smalls.tile([128, KC], FP32)
            nc.scalar.square(m2sq, m2a)
            v2 = smalls.tile([128, KC], FP32)
            nc.vector.tensor_sub(v2, qs, m2sq)
            nc.scalar.activation(v2, v2, AF.Sqrt, bias=eps_t, scale=1.0)
            nc.vector.reciprocal(v2, v2)   # rstd2
            a2a = smalls.tile([128, KC], FP32)
            nc.vector.tensor_mul(a2a, g7, v2)
            ra2 = smalls.tile([128, KC], FP32)
            nc.vector.reciprocal(ra2, a2a)
            nc.vector.tensor_mul(ra2, be7, ra2)
            nc.vector.tensor_sub(c2a, ra2, m2a)
            for kc in range(KC):
                if kc % 2 == 0:
                    nc.scalar.activation(
                        w8s[:, kc, :], w8t32[:, kc, :], AF.Copy,
                        scale=a2a[:, kc : kc + 1],
                    )
                else:
                    nc.vector.tensor_scalar_mul(
                        w8s[:, kc, :], w8t32[:, kc, :], a2a[:, kc : kc + 1]
                    )

            # ---- phase D: mm2 -> BN2-relu -> mm3 -> sigmoid -> out ----
            # Software-pipelined by one group: mm3 for group g-1 is
            # emitted between the two mm2 half-blocks of group g, so
            # each mm3 cc-pass finds all its PSUM banks already drained
            # and the scheduler keeps the stationary-reuse runs intact.
            group_sizes = [G] * (n_chunks // G - 1) + [2, 1, 1]
            assert sum(group_sizes) == n_chunks
            dstate = {"dcnt": 0}

            def emit_mm2(ks, kcs, h2t):
                for kc in kcs:
                    for k in ks:
                        ps2 = pmm.tile(
                            [128, NW], FP32, tag="mm", name=f"ps2_{kc}_{k}"
                        )
                        nc.tensor.matmul(
                            ps2, w7s[:, ts(kc, 128)], h1[:, ts(k, NW)],
                            start=True, stop=True,
                        )
                        h2 = h2p.tile([128, NW], FP16, tag="h2", name=f"h2_{kc}_{k}")
                        if dstate["dcnt"] < 4 * (n_chunks - 6) and _pick_act(
                            dstate["dcnt"]
                        ):
                            nc.scalar.activation(
                                h2, ps2, AF.Relu,
                                bias=c2a[:, kc : kc + 1], scale=1.0,
                            )
                        else:
                            nc.vector.tensor_scalar(
                                h2, ps2, c2a[:, kc : kc + 1], 0.0, ALU.add, ALU.max
                            )
                        dstate["dcnt"] += 1
                        h2t[(kc, k)] = h2

            def emit_mm3(ks, h2t, cc):
                csz = CSZ[cc]
                pses = {}
                for k in ks:
                    pses[k] = ppse.tile(
                        [128, NW], FP32, tag="pt", name=f"pse_{cc}_{k}"
                    )
                for kc in range(KC):
                    for k in ks:
                        nc.tensor.matmul(
                            pses[k][:csz],
                            w8s[:, kc, ds(cc * 128, csz)],
                            h2t[(kc, k)],
                            start=(kc == 0), stop=(kc == KC - 1),
                        )
                for k in ks:
                    em = emp.tile([128, NW], FP16, tag="em", name=f"em_{cc}_{k}")
                    nc.scalar.activation(
                        em[:csz], pses[k][:csz], AF.Sigmoid,
                        bias=b8[:csz, cc : cc + 1], scale=1.0,
                    )
                    nc.sync.dma_start(
                        out=emt_d[ds(cc * 128, csz), ts(k, NW)], in_=em[:csz]
                    )

            PIPE_D = True
            if PIPE_D:
                prev = None
                k0 = 0
                for gs in group_sizes:
                    ks = list(range(k0, k0 + gs))
                    k0 += gs
                    h2t = {}
                    emit_mm2(ks, [0, 1], h2t)
                    if prev is not None:
                        emit_mm3(prev[0], prev[1], 0)
                    emit_mm2(ks, [2, 3], h2t)
                    if prev is not None:
                        emit_mm3(prev[0], prev[1], 1)
                    prev = (ks, h2t)
                emit_mm3(prev[0], prev[1], 0)
                emit_mm3(prev[0], prev[1], 1)
            else:
                k0 = 0
                for gs in group_sizes:
                    ks = list(range(k0, k0 + gs))
                    k0 += gs
                    h2t = {}
                    emit_mm2(ks, [0, 1, 2, 3], h2t)
                    emit_mm3(ks, h2t, 0)
                    emit_mm3(ks, h2t, 1)

    _dedup_ldweights(nc)
    nc.compile()
    return nc


_cached = {}


def _get_program(n_chunks=NCH):
    if n_chunks not in _cached:
        _cached[n_chunks] = build_program(n_chunks)
    return _cached[n_chunks]


def make_in_maps(inputs, n=N):
    z = np.asarray(inputs["z"], np.float32)[:n]
    W6 = np.asarray(inputs["W6"], np.float32)
    g6 = np.asarray(inputs["g6"], np.float32)
    be6 = np.asarray(inputs["be6"], np.float32)
    W7 = np.asarray(inputs["W7"], np.float32)
    g7 = np.asarray(inputs["g7"], np.float32)
    be7 = np.asarray(inputs["be7"], np.float32)
    W8 = np.asarray(inputs["W8"], np.float32)
    b8 = np.asarray(inputs["b8"], np.float32)
    ztaug = np.ascontiguousarray(z.T.astype(np.float16))
    # z4[16q+z, k*128+nn] = zT[z, k*512+q*128+nn]
    z4 = np.ascontiguousarray(
        ztaug.reshape(ZD, n // NW, 4, 128)
        .transpose(2, 0, 1, 3)
        .reshape(4 * ZD, n // 4)
    )
    in_maps = []
    for p in range(P):
        w6b = np.zeros((97, NW), np.float16)
        for q in range(4):
            w6b[16 * q : 16 * q + ZD, 128 * q : 128 * (q + 1)] = (
                W6[p].T.astype(np.float16)
            )
        in_maps.append(
            {
                "ztaug": ztaug,
                "z4": z4,
                "w6b": w6b,
                "w6t": np.ascontiguousarray(W6[p].T.astype(np.float16)),
                "w7t": np.ascontiguousarray(W7[p].T),
                "w8t": np.ascontiguousarray(W8[p].T.reshape(KC, 128, CH)),
                "g6": np.ascontiguousarray(g6[p].reshape(F1, 1)),
                "be6": np.ascontiguousarray(be6[p].reshape(F1, 1)),
                "g7": np.ascontiguousarray(g7[p].reshape(KC, 128, 1)),
                "be7": np.ascontiguousarray(be7[p].reshape(KC, 128, 1)),
                "b8": np.ascontiguousarray(b8[p].reshape(CH, 1)),
            }
        )
    return in_maps


last_results = None


def kernel(**inputs):
    global last_results
    from concourse.bass_utils import run_bass_kernel_spmd

    nc = _get_program()
    in_maps = make_in_maps(inputs)
    res = run_bass_kernel_spmd(nc, in_maps, core_ids=list(range(P)))
    last_results = res
    out = np.empty((N, P, CH), np.float32)
    for p in range(P):
        out[:, p, :] = res.results[p]["emt"].T.astype(np.float32)
    return out
